# revision 12
# baseline (speedup 1.0000x reference)
"""TRN2 Bass kernel for the ConceptualMambaBlock problem (bf16 redesign).

Math (reference):
    x: [B=4, T=96, N=512, H=128] f32
    expanded = x @ W_exp.T + b_exp            # [B,T,N,2H]
    primary, gating = split(expanded, 2, -1)
    s_t = 0.9*s_{t-1} + 0.1*gating_t          # EMA along T
    out = (primary * sigmoid(s)) @ W_con.T + b_con

Restructure vs the fp32 baseline: the EMA is linear, so it commutes with
the gating Linear:

    s_t = (0.1*W_g) @ z_t + b_g*(1 - 0.9^t),   z_t = sum_{k<=t} 0.9^{t-k} x_k

  - The scan runs on the *input* x in SBUF at bf16 (batched 4-block DVE
    ops) instead of on the matmul output in PSUM at fp32.
  - The b_g*(1-0.9^t) term: +b_g rides the sigmoid's bias port; the
    -b_g*0.9^t part is injected through the scan via a "phantom column"
    per node: each node's 96 columns become 97, with column 0 holding
    the constant c = -10*Wg^-1*b_g (computed on host).  The scan reset
    lands on it (z_ph = c, z_1 = 0.9c + x_1, ...), so z carries an
    extra 0.9^t*c that maps to exactly -b_g*0.9^t through the gating
    matmul.  Both matmuls skip the phantom column via strided APs, so
    the primary path still sees raw x.  Zero extra instructions.
  - The 0.9^t weights inside the scan use a greedy per-column bf16
    multiplier sequence whose running products track 0.9^j to within
    one bf16 ulp (no compounding of the bf16 rounding of 0.9).

Everything runs in bf16 (PSUM/scan-state stay fp32 in HW).  Tolerance
is 2e-2; measured error ~4e-3.

Execution is organized in 4-block SUPER-ITERATIONS (iters 2S, 2S+1 of
2 blocks each) so that:
  - PE emits an 8-matmul back-to-back burst (4x mm1p + 4x mm2 of the
    previous super-iteration, whose inputs are long ready) — a >3.4us
    continuous burst that pushes the PE HAM to K=8/8 (2.4 GHz), and
    steady-state gaps stay under the ~3.4us re-throttle window.
  - mm2 consumes the PREVIOUS super-iteration's gate output, so the
    GPSIMD gate path's latency never blocks the in-order PE queue.
  - The sigmoid and the output Identity+bias each run as ONE 4-block
    op over PSUM banks 0-3.
Gate multiply y = (pp+b1p)*sig: even iters fused on DVE
(scalar_tensor_tensor from PSUM); odd iters drained by ACT (Identity+
b1p) and multiplied on the otherwise-idle GPSIMD (plain tensor_tensor,
the only elementwise op its Q7 ucode supports) — this offloads half of
the non-scan DVE work since the serial scan (~2.2 cyc/elem,
DVE-only) makes DVE the scarce engine.

PSUM (8 banks of [128, 512] f32), one manually-managed tile:
  banks {0,1}/{2,3} : pg for iteration parity 0/1; po(g) reuses the
                      parity-(g%2) pair after the joint sigmoid drained
                      it; freed by the joint outcopy.
  banks {4,5}/{6,7} : pp for iteration parity 0/1; freed by the gate
                      drain (stt on DVE / Identity on ACT).
DMA: input x via the sync HWDGE ring, output via the scalar HWDGE ring,
8-block groups (~790 KB); the gpsimd queue only runs gate multiplies.
"""

import numpy as np
import ml_dtypes

import concourse.bacc as bacc
import concourse.mybir as mybir
import concourse.tile as tile
from concourse.bass_utils import run_bass_kernel_spmd

F32 = mybir.dt.float32
BF16 = mybir.dt.bfloat16
AF = mybir.ActivationFunctionType
ALU = mybir.AluOpType

B, T, N, H = 4, 96, 512, 128
NCORES = 8
NLOC = N // 2          # 256 nodes per core
NB = 4                 # nodes per block
TP = T + 1             # 97 cols per node incl. phantom
TOK = NB * T           # 384 real columns per block
TOKX = NB * TP         # 388 stored columns per block
NBLK = NLOC // NB      # 64 blocks per core
SG = 8                 # blocks per DMA/scan group
NSG = NBLK // SG       # 8 groups
MG = 2                 # blocks per iteration
NMG = NBLK // MG       # 32 iterations
NSUP = NMG // 2        # 16 super-iterations
SCH = 4                # blocks per scan chunk

_NC_CACHE = None


def _greedy_mask_pattern():
    """Per-node TP-column multiplier sequence: col 0 is 0.0 (reset, lands
    on the phantom c column); col j (j=1..96) is a bf16 value m_j chosen
    so prod(m_1..m_j) tracks 0.9^j to within one bf16 ulp."""
    ms = [0.0]
    c_act = 1.0
    tgt = 1.0
    for _ in range(T):
        tgt *= 0.9
        m = float(np.asarray(tgt / c_act, dtype=np.float32).astype(ml_dtypes.bfloat16))
        ms.append(m)
        c_act *= m
    return np.array(ms, dtype=np.float64)


def _build():
    nc = bacc.Bacc()

    xt_h = nc.dram_tensor("xt", [H, NBLK, NB, TP], BF16, kind="ExternalInput")
    wpack_h = nc.dram_tensor("wpack", [H, 3 * H], BF16, kind="ExternalInput")
    bias_h = nc.dram_tensor("bias", [H, 3], F32, kind="ExternalInput")
    mask_h = nc.dram_tensor("mask", [H, SCH * TOKX], BF16, kind="ExternalInput")
    out_h = nc.dram_tensor("out", [H, NBLK, TOK], BF16, kind="ExternalOutput")

    with tile.TileContext(nc) as tc:
        with (
            tc.tile_pool(name="consts", bufs=1) as cp,
            tc.tile_pool(name="io", bufs=1) as io,
            tc.tile_pool(name="mid", bufs=1) as mid,
            tc.tile_pool(name="ps", bufs=1, space="PSUM") as ps,
        ):
            xts = [None] * NSG
            zs = [None] * NSG
            obs = [None] * NSG

            def load_group(s):
                xg = io.tile([H, SG, NB, TP], BF16, tag="x", name=f"x{s}", bufs=3)
                nc.sync.dma_start(out=xg[:], in_=xt_h[:, s * SG : (s + 1) * SG, :, :])
                xts[s] = xg

            load_group(0)  # x(0) first on the sync ring

            wpack_sb = cp.tile([H, 3 * H], BF16, tag="wpack")
            nc.sync.dma_start(out=wpack_sb[:], in_=wpack_h[:, :])
            bias_sb = cp.tile([H, 3], F32, tag="bias")
            nc.sync.dma_start(out=bias_sb[:], in_=bias_h[:, :])
            mask_sb = cp.tile([H, SCH * TOKX], BF16, tag="mask")
            nc.sync.dma_start(out=mask_sb[:], in_=mask_h[:, :])

            load_group(1)

            w1pT = wpack_sb[:, 0:H]
            w1gT = wpack_sb[:, H : 2 * H]
            wcT = wpack_sb[:, 2 * H : 3 * H]
            bg_ap = bias_sb[:, 0:1]
            b1p_ap = bias_sb[:, 1:2]
            b2_ap = bias_sb[:, 2:3]

            psum = ps.tile([H, 8, 512], F32, tag="all")

            # PE warm-up burst during the initial DMA/scan ramp
            for _ in range(10):
                nc.tensor.matmul(
                    psum[:, 6, 0:TOK], lhsT=wpack_sb[:, 0:H],
                    rhs=mask_sb[:, 0:TOK], start=True, stop=True,
                )

            def scan_chunk(s, h):
                if zs[s] is None:
                    zs[s] = mid.tile([H, SG, NB, TP], BF16, tag="z", name=f"z{s}", bufs=2)
                x2 = xts[s][:, h * SCH : (h + 1) * SCH, :, :].rearrange("p a b c -> p (a b c)")
                z2 = zs[s][:, h * SCH : (h + 1) * SCH, :, :].rearrange("p a b c -> p (a b c)")
                nc.vector.tensor_tensor_scan(
                    out=z2, data0=mask_sb[:], data1=x2,
                    initial=0.0, op0=ALU.mult, op1=ALU.add,
                )

            scan_chunk(0, 0)
            scan_chunk(0, 1)

            # carried state between super-iterations: gate outputs y for
            # iters 2S, 2S+1, to be consumed by mm2 in super-iter S+1
            prev = None  # (y_even, y_odd)

            def emit_mm2_out(S):
                # mm2 + joint outcopy for super-iter S's iterations, using
                # prev (y_even, y_odd).  po(2S) -> banks {0,1},
                # po(2S+1) -> {2,3}; both freed by the joint sigmoid of
                # the CURRENT super-iter before this runs.
                y_e, y_o = prev
                for j in range(MG):
                    nc.tensor.matmul(
                        psum[:, j, 0:TOK], lhsT=wcT, rhs=y_e[:, j, :],
                        start=True, stop=True,
                    )
                for j in range(MG):
                    nc.tensor.matmul(
                        psum[:, 2 + j, 0:TOK], lhsT=wcT, rhs=y_o[:, j, :],
                        start=True, stop=True,
                    )
                s1 = 4 * S // SG
                q1 = (4 * S) % SG
                nc.scalar.activation(
                    obs[s1][:, q1 : q1 + 4, :],
                    psum[:, 0:4, 0:TOK],
                    AF.Identity, bias=b2_ap, scale=1.0,
                )
                if q1 == 4:
                    nc.scalar.dma_start(
                        out=out_h[:, s1 * SG : (s1 + 1) * SG, :], in_=obs[s1][:]
                    )

            for S in range(NSUP):
                s = 4 * S // SG           # current 8-block group
                half = (4 * S) % SG // 4  # 0: first half of group, 1: second
                if half == 0:
                    if s + 2 < NSG:
                        load_group(s + 2)
                    obs[s] = io.tile([H, SG, TOK], BF16, tag="ob", name=f"ob{s}", bufs=2)

                zg, xg = zs[s], xts[s]
                b0 = half * 4             # first block of this super-iter in group

                # PE: 4x gating matmuls -> pg banks 0-3
                for j in range(4):
                    nc.tensor.matmul(
                        psum[:, j, 0:TOK], lhsT=w1gT,
                        rhs=zg[:, b0 + j, :, 1:TP],
                        start=True, stop=True,
                    )

                # DVE: scan chunk for the next group (no deps; keeps DVE
                # busy from the start of the super-iteration)
                if s + 1 < NSG:
                    scan_chunk(s + 1, half)

                # ACT: ONE 4-block sigmoid over banks 0-3
                sgq = mid.tile([H, 4, TOK], BF16, tag="sg", name=f"sg{S % 2}", bufs=2)
                nc.scalar.activation(
                    sgq[:], psum[:, 0:4, 0:TOK],
                    AF.Sigmoid, bias=bg_ap, scale=1.0,
                )

                # PE: 8-matmul burst: 4x mm1p then 4x mm2 of S-1 (y ready)
                for j in range(4):
                    nc.tensor.matmul(
                        psum[:, 4 + j, 0:TOK], lhsT=w1pT,
                        rhs=xg[:, b0 + j, :, 1:TP],
                        start=True, stop=True,
                    )

                # ACT: drain pp of the odd iteration -> bf16 (GP path);
                # emitted before the outcopy so the GPSIMD multiply starts
                # as early as possible
                ppb = mid.tile([H, MG, TOK], BF16, tag="ppb", name="ppb", bufs=2)
                nc.scalar.activation(
                    ppb[:], psum[:, 6:8, 0:TOK],
                    AF.Identity, bias=b1p_ap, scale=1.0,
                )

                if prev is not None:
                    emit_mm2_out(S - 1)

                # DVE: fused gate for the even iteration
                y_e = mid.tile([H, MG, TOK], BF16, tag="y", name=f"ye{S % 2}", bufs=2)
                nc.vector.scalar_tensor_tensor(
                    out=y_e[:], in0=psum[:, 4:6, 0:TOK], scalar=b1p_ap,
                    in1=sgq[:, 0:2, :], op0=ALU.add, op1=ALU.mult,
                )

                # GP: gate multiply for the odd iteration
                y_o = mid.tile([H, MG, TOK], BF16, tag="yo", name=f"yo{S % 2}", bufs=2)
                nc.gpsimd.tensor_tensor(
                    out=y_o[:].rearrange("p a b -> p (a b)"),
                    in0=ppb[:].rearrange("p a b -> p (a b)"),
                    in1=sgq[:, 2:4, :].rearrange("p a b -> p (a b)"),
                    op=ALU.mult,
                )
                prev = (y_e, y_o)

            emit_mm2_out(NSUP - 1)

    nc.finalize()
    return nc


def _get_nc():
    global _NC_CACHE
    if _NC_CACHE is None:
        _NC_CACHE = _build()
    return _NC_CACHE


def _inj_vector(W_exp, b_exp):
    """c with (0.1*Wg) @ c == -b_g: the phantom-column payload."""
    Wg = W_exp[H:].astype(np.float64)
    bg = b_exp[H:].astype(np.float64)
    try:
        c = -10.0 * np.linalg.solve(Wg, bg)
        if not np.all(np.isfinite(c)) or np.abs(c).max() > 2000.0:
            raise np.linalg.LinAlgError
    except np.linalg.LinAlgError:
        c = -10.0 * np.linalg.lstsq(Wg, bg, rcond=1e-2)[0]
    return c


def _in_maps(x, W_exp, b_exp, W_con, b_con):
    bf16 = ml_dtypes.bfloat16
    wpack = np.concatenate(
        [W_exp[:H, :].T, (0.1 * W_exp[H:, :]).T, W_con.T], axis=1
    ).astype(bf16)
    wpack = np.ascontiguousarray(wpack)

    bias = np.stack([b_exp[H:], b_exp[:H], b_con], axis=1).astype(np.float32)
    bias = np.ascontiguousarray(bias)

    mpat = _greedy_mask_pattern()                    # [97]
    mask = np.tile(mpat, SCH * NB)[None, :].repeat(H, axis=0).astype(bf16)
    mask = np.ascontiguousarray(mask)

    c = _inj_vector(W_exp, b_exp)

    maps = []
    for c_id in range(NCORES):
        bb, nh = c_id // 2, c_id % 2
        xs = x[bb, :, nh * NLOC : (nh + 1) * NLOC, :]  # [T, NLOC, H]
        xT = xs.transpose(2, 1, 0)                     # [H, NLOC, T]
        xhat = np.empty((H, NLOC, TP), dtype=np.float64)
        xhat[:, :, 0] = c[:, None]
        xhat[:, :, 1:] = xT
        maps.append(
            {
                "xt": np.ascontiguousarray(xhat.astype(bf16)).reshape(H, NBLK, NB, TP),
                "wpack": wpack,
                "bias": bias,
                "mask": mask,
            }
        )
    return maps


def run_spmd(x, W_exp, b_exp, W_con, b_con, **spmd_kwargs):
    """Run the 8-core kernel; returns (full_output, BassKernelResults)."""
    maps = _in_maps(x, W_exp, b_exp, W_con, b_con)
    res = run_bass_kernel_spmd(
        _get_nc(), maps, core_ids=list(range(NCORES)), **spmd_kwargs
    )
    out = np.empty((B, T, N, H), dtype=np.float32)
    for c_id in range(NCORES):
        bb, nh = c_id // 2, c_id % 2
        oT = res.results[c_id]["out"].astype(np.float32).reshape(H, NLOC, T)
        out[bb, :, nh * NLOC : (nh + 1) * NLOC, :] = oT.transpose(2, 1, 0)
    return out, res


def kernel(spatial_temporal_representation, W_exp, b_exp, W_con, b_con):
    out, _ = run_spmd(
        np.asarray(spatial_temporal_representation, dtype=np.float32),
        np.asarray(W_exp, dtype=np.float32),
        np.asarray(b_exp, dtype=np.float32),
        np.asarray(W_con, dtype=np.float32),
        np.asarray(b_con, dtype=np.float32),
    )
    return out


# revision 14
# speedup vs baseline: 1.0824x; 1.0824x over previous
"""TRN2 Bass kernel for the ConceptualMambaBlock problem (bf16 redesign).

Math (reference):
    x: [B=4, T=96, N=512, H=128] f32
    expanded = x @ W_exp.T + b_exp            # [B,T,N,2H]
    primary, gating = split(expanded, 2, -1)
    s_t = 0.9*s_{t-1} + 0.1*gating_t          # EMA along T
    out = (primary * sigmoid(s)) @ W_con.T + b_con

Restructure vs the fp32 baseline: the EMA is linear, so it commutes with
the gating Linear:

    s_t = (0.1*W_g) @ z_t + b_g*(1 - 0.9^t),   z_t = sum_{k<=t} 0.9^{t-k} x_k

  - The scan runs on the *input* x in SBUF at bf16 (4-block batched DVE
    ops) instead of on the matmul output in PSUM at fp32: both cheaper
    per element and independent of the matmul pipeline (pure prefetch).
  - The b_g*(1-0.9^t) term: +b_g rides the sigmoid's bias port; the
    -b_g*0.9^t part is injected through the scan via a "phantom column"
    per node: each node's 96 columns become 97, with column 0 holding
    the constant c = -10*Wg^-1*b_g (computed on host).  The scan reset
    lands on it (z_ph = c, z_1 = 0.9c + x_1, ...), so z carries an
    extra 0.9^t*c that maps to exactly -b_g*0.9^t through the gating
    matmul.  Both matmuls skip the phantom column via strided APs, so
    the primary path still sees raw x.  Zero extra instructions.
  - The 0.9^t weights inside the scan use a greedy per-column bf16
    multiplier sequence whose running products track 0.9^j to within
    one bf16 ulp (no compounding of the bf16 rounding of 0.9).

Everything runs in bf16 (PSUM and the scan state stay fp32 in HW):
PE matmuls at the bf16 rate, input+output HBM traffic halved.
Tolerance is 2e-2; measured error ~4e-3.

Pipeline: fine-grained 2-block iterations (one iteration of slack on
every cross-engine edge), mirroring the well-overlapped fp32 baseline:
  PE  g: mm1g(g) x2 -> pg | mm1p(g) x2 -> pp | mm2(g-1) x2 -> po
  ACT g: sigmoid(g) [pg -> bf16 gate] | Identity+b2 outcopy(g-1)
  DVE g: scan chunk (next group, no deps - emitted first so it fills
         any wait) | stt(g-1): y = (pp+b1p)*sig fused PSUM drain
PSUM banks (8 x [128,512] f32, one manually-managed tile):
  {0,1}/{2,3}: pg, iteration parity; freed by the sigmoid (2-iter reuse)
  {4,5}/{6,7}: pp, iteration parity; po(g-1) reuses the pair after the
               stt drain, freed by the outcopy.
The PE HAM starts throttled (K=4/8, 1.2 GHz): a prologue warm-up burst
plus one 8-matmul dummy burst after the pipeline is primed (iteration
2) push it to K=8/8; the steady-state PE gaps stay below the ~3.4us
re-throttle window, so it remains warm.
DMA: input via the sync HWDGE ring, output via the scalar HWDGE ring,
8-block groups (~790 KB).
"""

import numpy as np
import ml_dtypes

import concourse.bacc as bacc
import concourse.mybir as mybir
import concourse.tile as tile
from concourse.bass_utils import run_bass_kernel_spmd

F32 = mybir.dt.float32
BF16 = mybir.dt.bfloat16
AF = mybir.ActivationFunctionType
ALU = mybir.AluOpType

B, T, N, H = 4, 96, 512, 128
NCORES = 8
NLOC = N // 2          # 256 nodes per core
NB = 4                 # nodes per block
TP = T + 1             # 97 cols per node incl. phantom
TOK = NB * T           # 384 real columns per block
TOKX = NB * TP         # 388 stored columns per block
NBLK = NLOC // NB      # 64 blocks per core
SG = 8                 # blocks per DMA/scan group
NSG = NBLK // SG       # 8 groups
MG = 2                 # blocks per iteration
NMG = NBLK // MG       # 32 iterations
SCH = 4                # blocks per scan chunk (2 chunks per group)

_NC_CACHE = None


def _greedy_mask_pattern():
    """Per-node TP-column multiplier sequence: col 0 is 0.0 (reset, lands
    on the phantom c column); col j (j=1..96) is a bf16 value m_j chosen
    so prod(m_1..m_j) tracks 0.9^j to within one bf16 ulp."""
    ms = [0.0]
    c_act = 1.0
    tgt = 1.0
    for _ in range(T):
        tgt *= 0.9
        m = float(np.asarray(tgt / c_act, dtype=np.float32).astype(ml_dtypes.bfloat16))
        ms.append(m)
        c_act *= m
    return np.array(ms, dtype=np.float64)


def _build():
    nc = bacc.Bacc()

    xt_h = nc.dram_tensor("xt", [H, NBLK, NB, TP], BF16, kind="ExternalInput")
    wpack_h = nc.dram_tensor("wpack", [H, 3 * H], BF16, kind="ExternalInput")
    bias_h = nc.dram_tensor("bias", [H, 3], F32, kind="ExternalInput")
    mask_h = nc.dram_tensor("mask", [H, SCH * TOKX], BF16, kind="ExternalInput")
    out_h = nc.dram_tensor("out", [H, NBLK, TOK], BF16, kind="ExternalOutput")

    with tile.TileContext(nc) as tc:
        with (
            tc.tile_pool(name="consts", bufs=1) as cp,
            tc.tile_pool(name="io", bufs=1) as io,
            tc.tile_pool(name="mid", bufs=1) as mid,
            tc.tile_pool(name="ps", bufs=1, space="PSUM") as ps,
        ):
            state = {}
            xts = [None] * NSG
            zs = [None] * NSG
            obs = [None] * NSG

            def load_group(s):
                xg = io.tile([H, SG, NB, TP], BF16, tag="x", name=f"x{s}", bufs=3)
                nc.sync.dma_start(out=xg[:], in_=xt_h[:, s * SG : (s + 1) * SG, :, :])
                xts[s] = xg

            load_group(0)  # x(0) first on the sync ring

            wpack_sb = cp.tile([H, 3 * H], BF16, tag="wpack")
            nc.sync.dma_start(out=wpack_sb[:], in_=wpack_h[:, :])
            bias_sb = cp.tile([H, 3], F32, tag="bias")
            nc.sync.dma_start(out=bias_sb[:], in_=bias_h[:, :])
            mask_sb = cp.tile([H, SCH * TOKX], BF16, tag="mask")
            nc.sync.dma_start(out=mask_sb[:], in_=mask_h[:, :])

            load_group(1)

            w1pT = wpack_sb[:, 0:H]
            w1gT = wpack_sb[:, H : 2 * H]
            wcT = wpack_sb[:, 2 * H : 3 * H]
            bg_ap = bias_sb[:, 0:1]
            b1p_ap = bias_sb[:, 1:2]
            b2_ap = bias_sb[:, 2:3]

            psum = ps.tile([H, 8, 512], F32, tag="all")

            # prologue warm-up: fills the initial DMA/scan wait with PE
            # activity (bank 6's first real use is ~2 iterations in)
            for _ in range(10):
                nc.tensor.matmul(
                    psum[:, 6, 0:TOK], lhsT=wpack_sb[:, 0:H],
                    rhs=mask_sb[:, 0:TOK], start=True, stop=True,
                )

            def scan_chunk(s, h):
                if zs[s] is None:
                    zs[s] = mid.tile([H, SG, NB, TP], BF16, tag="z", name=f"z{s}", bufs=2)
                x2 = xts[s][:, h * SCH : (h + 1) * SCH, :, :].rearrange("p a b c -> p (a b c)")
                z2 = zs[s][:, h * SCH : (h + 1) * SCH, :, :].rearrange("p a b c -> p (a b c)")
                nc.vector.tensor_tensor_scan(
                    out=z2, data0=mask_sb[:], data1=x2,
                    initial=0.0, op0=ALU.mult, op1=ALU.add,
                )

            scan_chunk(0, 0)
            scan_chunk(0, 1)

            def emit_stt(g):
                # y = (pp + b1p) * sg, fused PSUM drain on DVE
                k1 = g % 2
                y_t = mid.tile([H, MG, TOK], BF16, tag="y", name=f"y{g % 4}", bufs=3)
                nc.vector.scalar_tensor_tensor(
                    out=y_t[:],
                    in0=psum[:, 4 + 2 * k1 : 6 + 2 * k1, 0:TOK],
                    scalar=b1p_ap,
                    in1=state[g]["sg"][:],
                    op0=ALU.add, op1=ALU.mult,
                )
                state[g]["y"] = y_t

            def emit_mm2_and_out(g):
                # po reuses pp's parity pair (already drained by the stt)
                k1 = g % 2
                s1 = g * MG // SG
                q1 = (g * MG % SG) // MG
                y_t = state[g]["y"]
                for j in range(MG):
                    nc.tensor.matmul(
                        psum[:, 4 + 2 * k1 + j, 0:TOK], lhsT=wcT, rhs=y_t[:, j, :],
                        start=True, stop=True,
                    )
                nc.scalar.activation(
                    obs[s1][:, q1 * MG : (q1 + 1) * MG, :],
                    psum[:, 4 + 2 * k1 : 6 + 2 * k1, 0:TOK],
                    AF.Identity, bias=b2_ap, scale=1.0,
                )
                if q1 == SG // MG - 1:
                    nc.scalar.dma_start(
                        out=out_h[:, s1 * SG : (s1 + 1) * SG, :], in_=obs[s1][:]
                    )
                del state[g]

            for g in range(NMG):
                s = g * MG // SG
                q = (g * MG % SG) // MG
                k = g % 2
                if q == 0:
                    if s + 2 < NSG:
                        load_group(s + 2)
                    obs[s] = io.tile([H, SG, TOK], BF16, tag="ob", name=f"ob{s}", bufs=2)

                # DVE: scan prefetch first - no dependencies, so it fills
                # any wait ahead of the stt on the DVE FIFO
                if s + 1 < NSG and q in (1, 2):
                    scan_chunk(s + 1, q - 1)

                # PE: gating matmuls
                for j in range(MG):
                    nc.tensor.matmul(
                        psum[:, 2 * k + j, 0:TOK], lhsT=w1gT,
                        rhs=zs[s][:, q * MG + j, :, 1:TP],
                        start=True, stop=True,
                    )

                # ACT: sigmoid as early as possible
                sg_t = mid.tile([H, MG, TOK], BF16, tag="sg", name=f"sg{g % 4}", bufs=3)
                nc.scalar.activation(
                    sg_t[:], psum[:, 2 * k : 2 * k + 2, 0:TOK],
                    AF.Sigmoid, bias=bg_ap, scale=1.0,
                )

                # PE: primary matmuls
                for j in range(MG):
                    nc.tensor.matmul(
                        psum[:, 4 + 2 * k + j, 0:TOK], lhsT=w1pT,
                        rhs=xts[s][:, q * MG + j, :, 1:TP],
                        start=True, stop=True,
                    )

                # one-time HAM warm burst once the pipeline is primed: 8
                # back-to-back dummies (>3.4us cold) flip the PE to 2.4 GHz;
                # steady-state gaps are short enough to keep it there.
                # pg bank 0's next writer is 2 iterations away - no stall.
                if g == 2:
                    for _ in range(8):
                        nc.tensor.matmul(
                            psum[:, 0, 0:TOK], lhsT=wpack_sb[:, 0:H],
                            rhs=mask_sb[:, 0:TOK], start=True, stop=True,
                        )

                # DVE: previous iteration's fused gate drain
                if g - 1 in state:
                    emit_stt(g - 1)

                state[g] = {"sg": sg_t}

                # PE: mm2 + ACT outcopy + DMA of g-1
                if g - 1 in state and "y" in state[g - 1]:
                    emit_mm2_and_out(g - 1)

            emit_stt(NMG - 1)
            emit_mm2_and_out(NMG - 1)

    nc.finalize()
    return nc


def _get_nc():
    global _NC_CACHE
    if _NC_CACHE is None:
        _NC_CACHE = _build()
    return _NC_CACHE


def _inj_vector(W_exp, b_exp):
    """c with (0.1*Wg) @ c == -b_g: the phantom-column payload."""
    Wg = W_exp[H:].astype(np.float64)
    bg = b_exp[H:].astype(np.float64)
    try:
        c = -10.0 * np.linalg.solve(Wg, bg)
        if not np.all(np.isfinite(c)) or np.abs(c).max() > 2000.0:
            raise np.linalg.LinAlgError
    except np.linalg.LinAlgError:
        c = -10.0 * np.linalg.lstsq(Wg, bg, rcond=1e-2)[0]
    return c


def _in_maps(x, W_exp, b_exp, W_con, b_con):
    bf16 = ml_dtypes.bfloat16
    wpack = np.concatenate(
        [W_exp[:H, :].T, (0.1 * W_exp[H:, :]).T, W_con.T], axis=1
    ).astype(bf16)
    wpack = np.ascontiguousarray(wpack)

    bias = np.stack([b_exp[H:], b_exp[:H], b_con], axis=1).astype(np.float32)
    bias = np.ascontiguousarray(bias)

    mpat = _greedy_mask_pattern()                    # [97]
    mask = np.tile(mpat, SCH * NB)[None, :].repeat(H, axis=0).astype(bf16)
    mask = np.ascontiguousarray(mask)

    c = _inj_vector(W_exp, b_exp)

    maps = []
    for c_id in range(NCORES):
        bb, nh = c_id // 2, c_id % 2
        xs = x[bb, :, nh * NLOC : (nh + 1) * NLOC, :]  # [T, NLOC, H]
        xT = xs.transpose(2, 1, 0)                     # [H, NLOC, T]
        xhat = np.empty((H, NLOC, TP), dtype=np.float64)
        xhat[:, :, 0] = c[:, None]
        xhat[:, :, 1:] = xT
        maps.append(
            {
                "xt": np.ascontiguousarray(xhat.astype(bf16)).reshape(H, NBLK, NB, TP),
                "wpack": wpack,
                "bias": bias,
                "mask": mask,
            }
        )
    return maps


def run_spmd(x, W_exp, b_exp, W_con, b_con, **spmd_kwargs):
    """Run the 8-core kernel; returns (full_output, BassKernelResults)."""
    maps = _in_maps(x, W_exp, b_exp, W_con, b_con)
    res = run_bass_kernel_spmd(
        _get_nc(), maps, core_ids=list(range(NCORES)), **spmd_kwargs
    )
    out = np.empty((B, T, N, H), dtype=np.float32)
    for c_id in range(NCORES):
        bb, nh = c_id // 2, c_id % 2
        oT = res.results[c_id]["out"].astype(np.float32).reshape(H, NLOC, T)
        out[bb, :, nh * NLOC : (nh + 1) * NLOC, :] = oT.transpose(2, 1, 0)
    return out, res


def kernel(spatial_temporal_representation, W_exp, b_exp, W_con, b_con):
    out, _ = run_spmd(
        np.asarray(spatial_temporal_representation, dtype=np.float32),
        np.asarray(W_exp, dtype=np.float32),
        np.asarray(b_exp, dtype=np.float32),
        np.asarray(W_con, dtype=np.float32),
        np.asarray(b_con, dtype=np.float32),
    )
    return out


# revision 15
# speedup vs baseline: 1.3234x; 1.2227x over previous
"""TRN2 Bass kernel for the ConceptualMambaBlock problem (bf16 redesign).

Math (reference):
    x: [B=4, T=96, N=512, H=128] f32
    expanded = x @ W_exp.T + b_exp            # [B,T,N,2H]
    primary, gating = split(expanded, 2, -1)
    s_t = 0.9*s_{t-1} + 0.1*gating_t          # EMA along T
    out = (primary * sigmoid(s)) @ W_con.T + b_con

Restructure vs the fp32 baseline: the EMA is linear, so it commutes with
the gating Linear:

    s_t = (0.1*W_g) @ z_t + b_g*(1 - 0.9^t),   z_t = sum_{k<=t} 0.9^{t-k} x_k

  - The scan runs on the *input* x in SBUF at bf16 (4-block batched DVE
    ops) instead of on the matmul output in PSUM at fp32: both cheaper
    per element and independent of the matmul pipeline (pure prefetch).
  - The b_g*(1-0.9^t) term: +b_g rides the sigmoid's bias port; the
    -b_g*0.9^t part is injected through the scan via a "phantom column"
    per node: each node's 96 columns become 97, with column 0 holding
    the constant c = -10*Wg^-1*b_g (computed on host).  The scan reset
    lands on it (z_ph = c, z_1 = 0.9c + x_1, ...), so z carries an
    extra 0.9^t*c that maps to exactly -b_g*0.9^t through the gating
    matmul.  Both matmuls skip the phantom column via strided APs, so
    the primary path still sees raw x.  Zero extra instructions.
  - The 0.9^t weights inside the scan use a greedy per-column bf16
    multiplier sequence whose running products track 0.9^j to within
    one bf16 ulp (no compounding of the bf16 rounding of 0.9).

Everything runs in bf16 (PSUM and the scan state stay fp32 in HW):
PE matmuls at the bf16 rate, input+output HBM traffic halved.
Tolerance is 2e-2; measured error ~4e-3.

Pipeline: fine-grained 2-block iterations (one iteration of slack on
every cross-engine edge), mirroring the well-overlapped fp32 baseline:
  PE  g: mm1g(g) x2 -> pg | mm1p(g) x2 -> pp | mm2(g-1) x2 -> po
  ACT g: sigmoid(g) [pg -> bf16 gate] | Identity+b2 outcopy(g-1)
  DVE g: scan chunk (next group, no deps - emitted first so it fills
         any wait) | stt(g-1): y = (pp+b1p)*sig fused PSUM drain
PSUM banks (8 x [128,512] f32, one manually-managed tile):
  {0,1}/{2,3}: pg, iteration parity; freed by the sigmoid (2-iter reuse)
  {4,5}/{6,7}: pp, iteration parity; po(g-1) reuses the pair after the
               stt drain, freed by the outcopy.
The PE HAM starts throttled (K=4/8, 1.2 GHz): a prologue warm-up burst
plus one 8-matmul dummy burst after the pipeline is primed (iteration
2) push it to K=8/8; the steady-state PE gaps stay below the ~3.4us
re-throttle window, so it remains warm.
DMA: input via the sync HWDGE ring, output via the scalar HWDGE ring,
8-block groups (~790 KB).
"""

import numpy as np
import ml_dtypes

import concourse.bacc as bacc
import concourse.mybir as mybir
import concourse.tile as tile
from concourse.bass_utils import run_bass_kernel_spmd

F32 = mybir.dt.float32
BF16 = mybir.dt.bfloat16
AF = mybir.ActivationFunctionType
ALU = mybir.AluOpType

B, T, N, H = 4, 96, 512, 128
NCORES = 8
NLOC = N // 2          # 256 nodes per core
NB = 4                 # nodes per block
TP = T + 1             # 97 cols per node incl. phantom
TOK = NB * T           # 384 real columns per block
TOKX = NB * TP         # 388 stored columns per block
NBLK = NLOC // NB      # 64 blocks per core
SG = 8                 # blocks per DMA/scan group
NSG = NBLK // SG       # 8 groups
MG = 2                 # blocks per iteration
NMG = NBLK // MG       # 32 iterations
# scan runs as 1-block chunks (2 per iteration, kept 2 blocks ahead)
# so the long serial scan never delays the stt->mm2 chain on the DVE FIFO

_NC_CACHE = None


def _greedy_mask_pattern():
    """Per-node TP-column multiplier sequence: col 0 is 0.0 (reset, lands
    on the phantom c column); col j (j=1..96) is a bf16 value m_j chosen
    so prod(m_1..m_j) tracks 0.9^j to within one bf16 ulp."""
    ms = [0.0]
    c_act = 1.0
    tgt = 1.0
    for _ in range(T):
        tgt *= 0.9
        m = float(np.asarray(tgt / c_act, dtype=np.float32).astype(ml_dtypes.bfloat16))
        ms.append(m)
        c_act *= m
    return np.array(ms, dtype=np.float64)


def _build():
    nc = bacc.Bacc()

    xt_h = nc.dram_tensor("xt", [H, NBLK, NB, TP], BF16, kind="ExternalInput")
    wpack_h = nc.dram_tensor("wpack", [H, 3 * H], BF16, kind="ExternalInput")
    bias_h = nc.dram_tensor("bias", [H, 3], F32, kind="ExternalInput")
    mask_h = nc.dram_tensor("mask", [H, TOKX], BF16, kind="ExternalInput")
    out_h = nc.dram_tensor("out", [H, NBLK, TOK], BF16, kind="ExternalOutput")

    with tile.TileContext(nc) as tc:
        with (
            tc.tile_pool(name="consts", bufs=1) as cp,
            tc.tile_pool(name="io", bufs=1) as io,
            tc.tile_pool(name="mid", bufs=1) as mid,
            tc.tile_pool(name="ps", bufs=1, space="PSUM") as ps,
        ):
            state = {}
            xts = [None] * NSG
            zs = [None] * NSG
            obs = [None] * NSG

            def load_group(s):
                xg = io.tile([H, SG, NB, TP], BF16, tag="x", name=f"x{s}", bufs=3)
                nc.sync.dma_start(out=xg[:], in_=xt_h[:, s * SG : (s + 1) * SG, :, :])
                xts[s] = xg

            load_group(0)  # x(0) first on the sync ring

            wpack_sb = cp.tile([H, 3 * H], BF16, tag="wpack")
            nc.sync.dma_start(out=wpack_sb[:], in_=wpack_h[:, :])
            bias_sb = cp.tile([H, 3], F32, tag="bias")
            nc.sync.dma_start(out=bias_sb[:], in_=bias_h[:, :])
            mask_sb = cp.tile([H, TOKX], BF16, tag="mask")
            nc.sync.dma_start(out=mask_sb[:], in_=mask_h[:, :])

            load_group(1)

            w1pT = wpack_sb[:, 0:H]
            w1gT = wpack_sb[:, H : 2 * H]
            wcT = wpack_sb[:, 2 * H : 3 * H]
            bg_ap = bias_sb[:, 0:1]
            b1p_ap = bias_sb[:, 1:2]
            b2_ap = bias_sb[:, 2:3]

            psum = ps.tile([H, 8, 512], F32, tag="all")

            # prologue warm-up: fills the initial DMA/scan wait with PE
            # activity (bank 6's first real use is ~2 iterations in)
            for _ in range(10):
                nc.tensor.matmul(
                    psum[:, 6, 0:TOK], lhsT=wpack_sb[:, 0:H],
                    rhs=mask_sb[:, 0:TOK], start=True, stop=True,
                )

            def scan_block(b):
                s0, h = b // SG, b % SG
                if zs[s0] is None:
                    zs[s0] = mid.tile([H, SG, NB, TP], BF16, tag="z", name=f"z{s0}", bufs=2)
                x2 = xts[s0][:, h, :, :].rearrange("p b c -> p (b c)")
                z2 = zs[s0][:, h, :, :].rearrange("p b c -> p (b c)")
                nc.vector.tensor_tensor_scan(
                    out=z2, data0=mask_sb[:], data1=x2,
                    initial=0.0, op0=ALU.mult, op1=ALU.add,
                )

            for b in range(4):
                scan_block(b)

            def emit_stt(g):
                # y = (pp + b1p) * sg, fused PSUM drain on DVE
                k1 = g % 2
                y_t = mid.tile([H, MG, TOK], BF16, tag="y", name=f"y{g % 4}", bufs=3)
                nc.vector.scalar_tensor_tensor(
                    out=y_t[:],
                    in0=psum[:, 4 + 2 * k1 : 6 + 2 * k1, 0:TOK],
                    scalar=b1p_ap,
                    in1=state[g]["sg"][:],
                    op0=ALU.add, op1=ALU.mult,
                )
                state[g]["y"] = y_t

            def emit_mm2_and_out(g):
                # po reuses pp's parity pair (already drained by the stt)
                k1 = g % 2
                s1 = g * MG // SG
                q1 = (g * MG % SG) // MG
                y_t = state[g]["y"]
                for j in range(MG):
                    nc.tensor.matmul(
                        psum[:, 4 + 2 * k1 + j, 0:TOK], lhsT=wcT, rhs=y_t[:, j, :],
                        start=True, stop=True,
                    )
                nc.scalar.activation(
                    obs[s1][:, q1 * MG : (q1 + 1) * MG, :],
                    psum[:, 4 + 2 * k1 : 6 + 2 * k1, 0:TOK],
                    AF.Identity, bias=b2_ap, scale=1.0,
                )
                if q1 == SG // MG - 1:
                    nc.scalar.dma_start(
                        out=out_h[:, s1 * SG : (s1 + 1) * SG, :], in_=obs[s1][:]
                    )
                del state[g]

            for g in range(NMG):
                s = g * MG // SG
                q = (g * MG % SG) // MG
                k = g % 2
                if q == 0:
                    if s + 2 < NSG:
                        load_group(s + 2)
                    obs[s] = io.tile([H, SG, TOK], BF16, tag="ob", name=f"ob{s}", bufs=2)

                # PE: gating matmuls
                for j in range(MG):
                    nc.tensor.matmul(
                        psum[:, 2 * k + j, 0:TOK], lhsT=w1gT,
                        rhs=zs[s][:, q * MG + j, :, 1:TP],
                        start=True, stop=True,
                    )

                # ACT: sigmoid as early as possible
                sg_t = mid.tile([H, MG, TOK], BF16, tag="sg", name=f"sg{g % 4}", bufs=3)
                nc.scalar.activation(
                    sg_t[:], psum[:, 2 * k : 2 * k + 2, 0:TOK],
                    AF.Sigmoid, bias=bg_ap, scale=1.0,
                )

                # PE: primary matmuls
                for j in range(MG):
                    nc.tensor.matmul(
                        psum[:, 4 + 2 * k + j, 0:TOK], lhsT=w1pT,
                        rhs=xts[s][:, q * MG + j, :, 1:TP],
                        start=True, stop=True,
                    )

                # one-time HAM warm burst once the pipeline is primed: 8
                # back-to-back dummies (>3.4us cold) flip the PE to 2.4 GHz;
                # steady-state gaps are short enough to keep it there.
                # pg bank 0's next writer is 2 iterations away - no stall.
                if g == 2:
                    for _ in range(8):
                        nc.tensor.matmul(
                            psum[:, 0, 0:TOK], lhsT=wpack_sb[:, 0:H],
                            rhs=mask_sb[:, 0:TOK], start=True, stop=True,
                        )

                # DVE: previous iteration's fused gate drain, then two
                # 1-block scan chunks (keeps the scan exactly 2 blocks
                # ahead without ever blocking the gate drain)
                if g - 1 in state:
                    emit_stt(g - 1)
                for b in (4 + 2 * g, 5 + 2 * g):
                    if b < NBLK:
                        scan_block(b)

                state[g] = {"sg": sg_t}

                # PE: mm2 + ACT outcopy + DMA of g-1
                if g - 1 in state and "y" in state[g - 1]:
                    emit_mm2_and_out(g - 1)

            emit_stt(NMG - 1)
            emit_mm2_and_out(NMG - 1)

    nc.finalize()
    return nc


def _get_nc():
    global _NC_CACHE
    if _NC_CACHE is None:
        _NC_CACHE = _build()
    return _NC_CACHE


def _inj_vector(W_exp, b_exp):
    """c with (0.1*Wg) @ c == -b_g: the phantom-column payload."""
    Wg = W_exp[H:].astype(np.float64)
    bg = b_exp[H:].astype(np.float64)
    try:
        c = -10.0 * np.linalg.solve(Wg, bg)
        if not np.all(np.isfinite(c)) or np.abs(c).max() > 2000.0:
            raise np.linalg.LinAlgError
    except np.linalg.LinAlgError:
        c = -10.0 * np.linalg.lstsq(Wg, bg, rcond=1e-2)[0]
    return c


def _in_maps(x, W_exp, b_exp, W_con, b_con):
    bf16 = ml_dtypes.bfloat16
    wpack = np.concatenate(
        [W_exp[:H, :].T, (0.1 * W_exp[H:, :]).T, W_con.T], axis=1
    ).astype(bf16)
    wpack = np.ascontiguousarray(wpack)

    bias = np.stack([b_exp[H:], b_exp[:H], b_con], axis=1).astype(np.float32)
    bias = np.ascontiguousarray(bias)

    mpat = _greedy_mask_pattern()                    # [97]
    mask = np.tile(mpat, NB)[None, :].repeat(H, axis=0).astype(bf16)
    mask = np.ascontiguousarray(mask)

    c = _inj_vector(W_exp, b_exp)

    maps = []
    for c_id in range(NCORES):
        bb, nh = c_id // 2, c_id % 2
        xs = x[bb, :, nh * NLOC : (nh + 1) * NLOC, :]  # [T, NLOC, H]
        xT = xs.transpose(2, 1, 0)                     # [H, NLOC, T]
        xhat = np.empty((H, NLOC, TP), dtype=np.float64)
        xhat[:, :, 0] = c[:, None]
        xhat[:, :, 1:] = xT
        maps.append(
            {
                "xt": np.ascontiguousarray(xhat.astype(bf16)).reshape(H, NBLK, NB, TP),
                "wpack": wpack,
                "bias": bias,
                "mask": mask,
            }
        )
    return maps


def run_spmd(x, W_exp, b_exp, W_con, b_con, **spmd_kwargs):
    """Run the 8-core kernel; returns (full_output, BassKernelResults)."""
    maps = _in_maps(x, W_exp, b_exp, W_con, b_con)
    res = run_bass_kernel_spmd(
        _get_nc(), maps, core_ids=list(range(NCORES)), **spmd_kwargs
    )
    out = np.empty((B, T, N, H), dtype=np.float32)
    for c_id in range(NCORES):
        bb, nh = c_id // 2, c_id % 2
        oT = res.results[c_id]["out"].astype(np.float32).reshape(H, NLOC, T)
        out[bb, :, nh * NLOC : (nh + 1) * NLOC, :] = oT.transpose(2, 1, 0)
    return out, res


def kernel(spatial_temporal_representation, W_exp, b_exp, W_con, b_con):
    out, _ = run_spmd(
        np.asarray(spatial_temporal_representation, dtype=np.float32),
        np.asarray(W_exp, dtype=np.float32),
        np.asarray(b_exp, dtype=np.float32),
        np.asarray(W_con, dtype=np.float32),
        np.asarray(b_con, dtype=np.float32),
    )
    return out
